# revision 31
# baseline (speedup 1.0000x reference)
"""FFM layer (field-aware factorization machine) on 8 Trainium2 cores.

Strategy: data-parallel over batch (2048 samples/core). The embedding tables
are re-laid-out on the host into one row per global vocab id g (owned by
exactly one field c = g // V): the 19 *other* fields' embeddings for that id
in fp8 e3m4 (scaled by 32; values are ~N(0, 0.05^2) so 4-bit-mantissa fp8
at this scale keeps max output rel-err ~1e-3..1e-2, well inside the 2e-2
gate), plus the w_sparse value as bf16 (scaled by 32*32 so it sums in the
same accumulator as the pair products), padded to 512 B (two 256 B dma_gather
granules; 33% less gather traffic than the bf16/768 B layout).

The gather uses nc.gpsimd.dma_gather (int16 indices). Indices must fit int16,
so gathers address vocab windows of 3 fields (3*10000 < 32767), with
window-relative indices. Tiles are processed in groups of 2 (one gather per
(window, group)), single_packet=False so each row is its own packet and the
SDMA engines interleave rows across the 4 SWDGE queues (hides HBM read
latency; ~31 ns/row/engine vs ~38 with one big packet per engine).

Compute per group: 19 DVE tensor_tensor multiplies (one per smaller field i,
batched over the group's tiles via 4-D access patterns) write all pair
products into a [P, ngg, 3072] bf16 scratch; one DVE scalar_tensor_tensor
(two-tensor form — single-src DVE ops enter 2-port perf mode whose exclusive
SBUF port lock starves SWDGE descriptor generation) drops the 20 w_sparse
values in behind them. Per tile, one scalar-engine accumulate reduces
products+wsp to a single f32 per sample (the last group splits this between
ACT and a DVE tensor_reduce to shorten the tail), the tensor engine does the
dense linear part, and a final fused activation computes
sigmoid(acc/1024 + linear).
"""

import os
import sys

import numpy as np


def _import_concourse():
    try:
        import concourse  # noqa: F401
    except ImportError:
        for p in ("/opt/trn_rl_repo", "/root/.axon_site/_ro/trn_rl_repo"):
            if os.path.isdir(p) and p not in sys.path:
                sys.path.insert(0, p)
    import concourse.bass as bass  # noqa: F401
    import concourse.mybir as mybir  # noqa: F401
    import concourse.tile as tile  # noqa: F401
    from concourse import bass_utils  # noqa: F401

    return bass, mybir, tile, bass_utils


# Problem constants (hardcoded per contract)
F = 20          # sparse fields
V = 10000       # vocab per field
VTOT = F * V    # 200000
D = 16          # embed dim
B = 16384       # batch
DD = 13         # dense feature count
N_CORES = 8
P = 128         # SBUF partitions

BPC = B // N_CORES          # 2048 samples per core
N_TILES = BPC // P          # 16 tiles of 128 samples
ROWB = 512                  # gather row bytes (= fp8 elements)
EMB = (F - 1) * D           # 304 fp8 payload elements
WSPB = EMB                  # byte offset of the bf16 w_sparse slot
SCALE = 32.0                # host-side fp8 scale; products come out *1024
CLIP = 15.4375              # e3m4 max normal is 15.5 (inf above)
NPAIR_ELEMS = (F * (F - 1) // 2) * D  # 3040 pair-product elements per sample
NRED = NPAIR_ELEMS + F      # +20 w_sparse values reduced in the same pass
PRODW = 3072                # per-tile stride in the product scratch
COLS_PER_WIN = 3            # fields per gather window (3*V < int16 max)
SINGLE_PACKET = False
N_QUEUES = 4
GROUPS = [2] * 8            # tiles per gather group
NGMAX = max(GROUPS)

WINDOWS = [
    (c0, min(COLS_PER_WIN, F - c0)) for c0 in range(0, F, COLS_PER_WIN)
]
NW = len(WINDOWS)


def _patch_queue_lanes():
    """Make Tile assign DMASW sem lanes per SWDGE queue (lane 2q/2q+1 for
    queue q) — the runtime locks each lane to one queue, but stock Tile
    round-robins lanes obliviously."""
    import concourse.tile_sem_assignment as tsa

    if getattr(tsa, "_ffm_queue_patch", False):
        return
    import concourse.mybir as mybir

    orig = tsa.TileClockTick._assign_tick

    def patched(self, inst):
        q = getattr(inst, "queue_num", None)
        if (
            q is not None
            and isinstance(inst, tsa.DMAInst)
            and inst.engine == mybir.EngineType.Pool
        ):
            state = getattr(self, "_ffm_perq", None)
            if state is None:
                state = {}
                self._ffm_perq = state
            self.next_sw_dma_idx = 2 * q + state.get(q, 0)
            orig(self, inst)
            state[q] = state.get(q, 0) ^ 1
            return
        orig(self, inst)

    tsa.TileClockTick._assign_tick = patched
    tsa._ffm_queue_patch = True


def _build_program(n_tiles=N_TILES, vtot=VTOT, for_sim=False):
    bass, mybir, tile, _ = _import_concourse()
    _patch_queue_lanes()

    v = vtot // F
    gs = GROUPS

    import concourse.bacc as bacc

    # Bacc (not plain Bass): its compile() runs generate_event_semaphores,
    # which splits multi-semaphore waits into InstEventSemaphore prefixes —
    # TRN2 instructions can carry only one inline wait — and inserts the
    # GPSIMD ucode library loads that dma_gather needs.
    nc = bacc.Bacc(None, target_bir_lowering=False, debug=for_sim,
                   num_swdge_queues=N_QUEUES)

    f32 = mybir.dt.float32
    bf16 = mybir.dt.bfloat16
    f8 = mybir.dt.float8e3
    i16 = mybir.dt.int16
    mult = mybir.AluOpType.mult
    copy_f = mybir.ActivationFunctionType.Copy
    sigm_f = mybir.ActivationFunctionType.Sigmoid

    t2 = nc.dram_tensor("t2", [vtot, ROWB], f8, kind="ExternalInput")
    idx_cols = sum(ncw * 8 * g for g in gs for (c0, ncw) in WINDOWS)
    idxs = nc.dram_tensor("idxs", [P, idx_cols], i16, kind="ExternalInput")
    dense_d = nc.dram_tensor("dense", [DD + 1, n_tiles, P], f32, kind="ExternalInput")
    wvec_d = nc.dram_tensor("wvec", [DD + 1, 1], f32, kind="ExternalInput")
    out = nc.dram_tensor("out", [P, n_tiles], f32, kind="ExternalOutput")

    with tile.TileContext(nc) as tc:
        with (
            tc.tile_pool(name="const", bufs=1) as cpool,
            tc.tile_pool(name="gather", bufs=3) as gpool,
            tc.tile_pool(name="scratch", bufs=2) as spool,
            tc.tile_pool(name="accp", bufs=4) as apool,
            tc.tile_pool(name="psum", bufs=2, space="PSUM") as pspool,
        ):
            dense_sb = cpool.tile([DD + 1, n_tiles, P], f32)
            wvec_sb = cpool.tile([DD + 1, 1], f32)
            out_all = cpool.tile([P, n_tiles], f32)

            # per-group idx tiles (separate tiles, not slices of one tile —
            # Tile would otherwise make gather 0 wait on every idx DMA),
            # first group's first, so gather 0 isn't gated on the whole
            # index array
            idx_sbs = []
            idx_off0 = 0
            for g_i, ngg in enumerate(gs):
                gcols = ngg * 8 * F
                idx_g = cpool.tile([P, gcols], i16, tag=f"idx{g_i}")
                nc.sync.dma_start(
                    out=idx_g[:],
                    in_=idxs[:, idx_off0 : idx_off0 + gcols],
                )
                idx_sbs.append(idx_g)
                idx_off0 += gcols
            nc.sync.dma_start(out=dense_sb[:], in_=dense_d[:])
            nc.sync.dma_start(out=wvec_sb[:], in_=wvec_d[:])

            gather_seq = 0
            tile_base = 0
            for g_i, ngg in enumerate(gs):
                # gather output must be contiguous, so gall gets a tag per
                # group size; prod below is NGMAX-shaped and sliced (DVE APs
                # handle the stride). The ngg=2 groups are first and last —
                # far apart — so one buffer suffices for them; the ngg=4
                # groups triple-buffer so a group's gathers never stall on
                # compute two groups back.
                gall = gpool.tile([P, F, ngg, ROWB], f8, tag=f"gall{ngg}")
                idx_g = idx_sbs[g_i]
                # reversed window order: the descending-i compute consumes
                # windows last-to-first, so emitting w6 first lets compute
                # start after the first gather of the group lands. idx
                # columns are laid out in emission order within the group.
                idx_off = 0
                for wi, (c0, ncw) in reversed(list(enumerate(WINDOWS))):
                    nidx = ncw * ngg * P
                    ncols = ncw * 8 * ngg
                    nc.gpsimd.dma_gather(
                        gall[:, c0 : c0 + ncw, :, :].rearrange(
                            "p c n r -> p (c n) r"
                        ),
                        t2[c0 * v : (c0 + ncw) * v, :],
                        idx_g[:, idx_off : idx_off + ncols],
                        nidx,
                        nidx,
                        ROWB,
                        single_packet=SINGLE_PACKET,
                        queue_num=gather_seq % N_QUEUES,
                    )
                    gather_seq += 1
                    idx_off += ncols

                prod_f = spool.tile([P, NGMAX, PRODW], bf16, tag="prod")
                prod = prod_f[:, :ngg, :]

                # pair products for all of the group's tiles at once:
                # per smaller-field i, out[p, n, j, d] =
                #   row_i[block j] * row_j[block i]   (j > i)
                off = 0
                for i in reversed(range(F - 1)):
                    cnt = F - 1 - i
                    x = gall[:, i, :, i * D : EMB].rearrange(
                        "p n (c d) -> p n c d", d=D
                    )
                    y = gall[:, i + 1 : F, :, i * D : (i + 1) * D].rearrange(
                        "p c n d -> p n c d"
                    )
                    nc.vector.tensor_tensor(
                        out=prod[:, :, off : off + cnt * D].rearrange(
                            "p n (c d) -> p n c d", d=D
                        ),
                        in0=x,
                        in1=y,
                        op=mult,
                    )
                    off += cnt * D

                # w_sparse values (bf16, pre-scaled by 1024) behind the
                # products so one reduction covers cross + linear_sparse.
                # scalar_tensor_tensor, NOT tensor_scalar: single-src DVE ops
                # enter 2-port perf mode, which takes the SBUF port pair as an
                # exclusive lock and starves SWDGE descriptor generation
                # (gathers stall); two-tensor ops never contend.
                wsp = gall[:, :, :, WSPB : WSPB + 2].bitcast(bf16).rearrange(
                    "p c n one -> p n (c one)"
                )
                nc.vector.scalar_tensor_tensor(
                    out=prod[:, :, NPAIR_ELEMS:NRED],
                    in0=wsp,
                    scalar=1.0,
                    in1=wsp,
                    op0=mult,
                    op1=mybir.AluOpType.bypass,
                )

                last_group = g_i == len(gs) - 1
                accg = None
                if last_group and ngg > 1:
                    # tail trim: odd tiles reduce on DVE (tensor_reduce never
                    # enters 2-port mode, so it can't starve SWDGE) in
                    # parallel with the even tiles' serial ACT accumulates
                    accg = apool.tile([P, NGMAX], f32, tag="accg")
                    nc.vector.tensor_reduce(
                        out=accg[:, 1:ngg:2],
                        in_=prod[:, 1:ngg:2, :NRED],
                        axis=mybir.AxisListType.X,
                        op=mybir.AluOpType.add,
                    )

                for n in range(ngg):
                    tt = tile_base + n
                    lin = apool.tile([P, 1], f32, tag="lin")

                    if accg is not None and n % 2 == 1:
                        acc = accg[:, n : n + 1]
                    else:
                        acc_t = apool.tile([P, 1], f32, tag="acc")
                        acc = acc_t[:]
                        # cross sum + w_sparse sum: one ACT accumulate
                        # (in-place copy; ACT streams read-then-write)
                        nc.scalar.activation(
                            out=prod[:, n, :NRED],
                            in_=prod[:, n, :NRED],
                            func=copy_f,
                            accum_out=acc,
                        )

                    # linear dense + bias on the (idle) tensor engine
                    ps = pspool.tile([P, 1], f32)
                    nc.tensor.matmul(
                        out=ps[:],
                        lhsT=dense_sb[:, tt, :],
                        rhs=wvec_sb[:, :1],
                        start=True,
                        stop=True,
                    )
                    nc.scalar.copy(out=lin[:], in_=ps[:])

                    # sigmoid(acc/1024 + linear)
                    nc.scalar.activation(
                        out=out_all[:, tt : tt + 1],
                        in_=acc,
                        func=sigm_f,
                        scale=1.0 / (SCALE * SCALE),
                        bias=lin[:],
                    )
                tile_base += ngg

            nc.sync.dma_start(out=out[:], in_=out_all[:])

    nc.compile()
    return nc


_PROGRAM_CACHE = {}


def _get_program():
    if "nc" not in _PROGRAM_CACHE:
        _PROGRAM_CACHE["nc"] = _build_program()
    return _PROGRAM_CACHE["nc"]


def make_idx_array(sparse_core, n_tiles=N_TILES, v=V):
    """sparse_core: [BPC, F] local ids (< V). Returns [P, idx_cols] i16.

    Column layout mirrors _build_program: groups per GROUPS, windows
    emitted in reversed order, idx element i at [partition i%16 (replicated
    8x down), col i//16]; within a gather i = (c_local * ngg + n) * 128 + p.
    """
    gs = GROUPS
    spc = sparse_core.reshape(P, n_tiles, F)  # [p, tt, c], sample s = p*n_tiles+tt
    cols = []
    tile_base = 0
    for ngg in gs:
        for wi, (c0, ncw) in reversed(list(enumerate(WINDOWS))):
            vals = spc[:, tile_base : tile_base + ngg, c0 : c0 + ncw].transpose(
                2, 1, 0
            ).astype(np.int64)
            vals = vals + (np.arange(ncw, dtype=np.int64) * v)[:, None, None]
            flat = vals.reshape(-1).astype(np.int16)
            m = len(flat) // 16
            cols.append(np.tile(flat.reshape(m, 16).T, (8, 1)))  # [128, m]
        tile_base += ngg
    return np.ascontiguousarray(np.concatenate(cols, axis=1))


def _prep_inputs(dense_input, sparse_input, tables, w_dense, w_sparse, bias):
    import ml_dtypes

    dense_input = np.asarray(dense_input, dtype=np.float32)
    sparse_input = np.asarray(sparse_input)
    tables = np.asarray(tables, dtype=np.float32)
    w_dense = np.asarray(w_dense, dtype=np.float32)
    w_sparse = np.asarray(w_sparse, dtype=np.float32)
    bias = np.asarray(bias, dtype=np.float32)

    # T2[g] = [tables[t, g, :]*32 as e3m4 for t != g//V] ++ [w_sparse[g]*1024
    # as bf16] ++ pad
    t2u8 = np.zeros((VTOT, ROWB), dtype=np.uint8)
    for c in range(F):
        sl = slice(c * V, (c + 1) * V)
        sel = [t for t in range(F) if t != c]
        emb = tables[sel, sl, :].transpose(1, 0, 2).reshape(V, EMB)
        emb8 = np.clip(emb * SCALE, -CLIP, CLIP).astype(ml_dtypes.float8_e3m4)
        t2u8[sl, :EMB] = emb8.view(np.uint8)
        wspv = (w_sparse[sl, 0] * (SCALE * SCALE)).astype(ml_dtypes.bfloat16)
        t2u8[sl, WSPB : WSPB + 2] = wspv[:, None].view(np.uint8)
    t2 = t2u8.view(ml_dtypes.float8_e3m4)

    sparse_i = sparse_input.astype(np.int64).reshape(N_CORES, BPC, F)
    dense_aug = np.concatenate(
        [dense_input, np.ones((B, 1), dtype=np.float32)], axis=1
    ).reshape(N_CORES, P, N_TILES, DD + 1)
    waug = np.concatenate([w_dense[:, 0], bias]).astype(np.float32)
    wvec = waug.reshape(DD + 1, 1)

    in_maps = []
    for k in range(N_CORES):
        in_maps.append(
            {
                "t2": t2,
                "idxs": make_idx_array(sparse_i[k]),
                "dense": np.ascontiguousarray(dense_aug[k].transpose(2, 1, 0)),
                "wvec": wvec,
            }
        )
    return in_maps


def kernel(dense_input, sparse_input, tables, w_dense, w_sparse, bias, _trace=False):
    _, _, _, bass_utils = _import_concourse()

    nc = _get_program()
    in_maps = _prep_inputs(dense_input, sparse_input, tables, w_dense, w_sparse, bias)
    res = bass_utils.run_bass_kernel_spmd(
        nc, in_maps, core_ids=list(range(N_CORES)), trace=_trace
    )
    outs = [res.results[k]["out"].reshape(BPC) for k in range(N_CORES)]
    full = np.concatenate(outs).reshape(B, 1).astype(np.float32)
    if _trace:
        return full, res
    return full


# revision 32
# speedup vs baseline: 1.0861x; 1.0861x over previous
"""FFM layer (field-aware factorization machine) on 8 Trainium2 cores.

Strategy: data-parallel over batch (2048 samples/core). The embedding tables
are re-laid-out on the host into one row per global vocab id g (owned by
exactly one field c = g // V): the 19 *other* fields' embeddings for that id
in fp8 e3m4 (scaled by 32; values are ~N(0, 0.05^2) so 4-bit-mantissa fp8
at this scale keeps max output rel-err ~1e-3..1e-2, well inside the 2e-2
gate), plus the w_sparse value as bf16 (scaled by 32*32 so it sums in the
same accumulator as the pair products), padded to 512 B (two 256 B dma_gather
granules; 33% less gather traffic than the bf16/768 B layout).

The gather uses nc.gpsimd.dma_gather (int16 indices). Indices must fit int16,
so gathers address vocab windows of 3 fields (3*10000 < 32767), with
window-relative indices. Tiles are processed in groups of 2 (one gather per
(window, group)), single_packet=False so each row is its own packet and the
SDMA engines interleave rows across the 4 SWDGE queues (hides HBM read
latency; ~31 ns/row/engine vs ~38 with one big packet per engine).

Compute per group: 19 DVE tensor_tensor multiplies (one per smaller field i,
batched over the group's tiles via 4-D access patterns) write all pair
products into a [P, ngg, 3072] bf16 scratch; one DVE scalar_tensor_tensor
(two-tensor form — single-src DVE ops enter 2-port perf mode whose exclusive
SBUF port lock starves SWDGE descriptor generation) drops the 20 w_sparse
values in behind them. Per tile, one scalar-engine accumulate reduces
products+wsp to a single f32 per sample (the last group splits this between
ACT and a DVE tensor_reduce to shorten the tail), the tensor engine does the
dense linear part, and a final fused activation computes
sigmoid(acc/1024 + linear).
"""

import os
import sys

import numpy as np


def _import_concourse():
    try:
        import concourse  # noqa: F401
    except ImportError:
        for p in ("/opt/trn_rl_repo", "/root/.axon_site/_ro/trn_rl_repo"):
            if os.path.isdir(p) and p not in sys.path:
                sys.path.insert(0, p)
    import concourse.bass as bass  # noqa: F401
    import concourse.mybir as mybir  # noqa: F401
    import concourse.tile as tile  # noqa: F401
    from concourse import bass_utils  # noqa: F401

    return bass, mybir, tile, bass_utils


# Problem constants (hardcoded per contract)
F = 20          # sparse fields
V = 10000       # vocab per field
VTOT = F * V    # 200000
D = 16          # embed dim
B = 16384       # batch
DD = 13         # dense feature count
N_CORES = 8
P = 128         # SBUF partitions

BPC = B // N_CORES          # 2048 samples per core
N_TILES = BPC // P          # 16 tiles of 128 samples
ROWB = 512                  # gather row bytes (= fp8 elements)
EMB = (F - 1) * D           # 304 fp8 payload elements
WSPB = EMB                  # byte offset of the bf16 w_sparse slot
SCALE = 32.0                # host-side fp8 scale; products come out *1024
CLIP = 15.4375              # e3m4 max normal is 15.5 (inf above)
NPAIR_ELEMS = (F * (F - 1) // 2) * D  # 3040 pair-product elements per sample
NRED = NPAIR_ELEMS + F      # +20 w_sparse values reduced in the same pass
PRODW = 3072                # per-tile stride in the product scratch
COLS_PER_WIN = 3            # fields per gather window (3*V < int16 max)
SINGLE_PACKET = False
N_QUEUES = 4
GROUPS = [2] * 8            # tiles per gather group
NGMAX = max(GROUPS)

WINDOWS = [
    (c0, min(COLS_PER_WIN, F - c0)) for c0 in range(0, F, COLS_PER_WIN)
]
NW = len(WINDOWS)


def _patch_queue_lanes():
    """Make Tile assign DMASW sem lanes per SWDGE queue (lane 2q/2q+1 for
    queue q) — the runtime locks each lane to one queue, but stock Tile
    round-robins lanes obliviously."""
    import concourse.tile_sem_assignment as tsa

    if getattr(tsa, "_ffm_queue_patch", False):
        return
    import concourse.mybir as mybir

    orig = tsa.TileClockTick._assign_tick

    def patched(self, inst):
        q = getattr(inst, "queue_num", None)
        if (
            q is not None
            and isinstance(inst, tsa.DMAInst)
            and inst.engine == mybir.EngineType.Pool
        ):
            state = getattr(self, "_ffm_perq", None)
            if state is None:
                state = {}
                self._ffm_perq = state
            self.next_sw_dma_idx = 2 * q + state.get(q, 0)
            orig(self, inst)
            state[q] = state.get(q, 0) ^ 1
            return
        orig(self, inst)

    tsa.TileClockTick._assign_tick = patched
    tsa._ffm_queue_patch = True


def _build_program(n_tiles=N_TILES, vtot=VTOT, for_sim=False):
    bass, mybir, tile, _ = _import_concourse()
    _patch_queue_lanes()

    v = vtot // F
    gs = GROUPS

    import concourse.bacc as bacc

    # Bacc (not plain Bass): its compile() runs generate_event_semaphores,
    # which splits multi-semaphore waits into InstEventSemaphore prefixes —
    # TRN2 instructions can carry only one inline wait — and inserts the
    # GPSIMD ucode library loads that dma_gather needs.
    nc = bacc.Bacc(None, target_bir_lowering=False, debug=for_sim,
                   num_swdge_queues=N_QUEUES)

    f32 = mybir.dt.float32
    bf16 = mybir.dt.bfloat16
    f8 = mybir.dt.float8e3
    i16 = mybir.dt.int16
    mult = mybir.AluOpType.mult
    copy_f = mybir.ActivationFunctionType.Copy
    sigm_f = mybir.ActivationFunctionType.Sigmoid

    t2 = nc.dram_tensor("t2", [vtot, ROWB], f8, kind="ExternalInput")
    idx_cols = sum(ncw * 8 * g for g in gs for (c0, ncw) in WINDOWS)
    idxs = nc.dram_tensor("idxs", [P, idx_cols], i16, kind="ExternalInput")
    dense_d = nc.dram_tensor("dense", [DD + 1, n_tiles, P], f32, kind="ExternalInput")
    wvec_d = nc.dram_tensor("wvec", [DD + 1, 1], f32, kind="ExternalInput")
    out = nc.dram_tensor("out", [P, n_tiles], f32, kind="ExternalOutput")

    with tile.TileContext(nc) as tc:
        with (
            tc.tile_pool(name="const", bufs=1) as cpool,
            tc.tile_pool(name="gather", bufs=3) as gpool,
            tc.tile_pool(name="scratch", bufs=2) as spool,
            tc.tile_pool(name="accp", bufs=4) as apool,
            tc.tile_pool(name="psum", bufs=2, space="PSUM") as pspool,
        ):
            dense_sb = cpool.tile([DD + 1, n_tiles, P], f32)
            wvec_sb = cpool.tile([DD + 1, 1], f32)
            out_all = cpool.tile([P, n_tiles], f32)

            # per-group idx tiles (separate tiles, not slices of one tile —
            # Tile would otherwise make gather 0 wait on every idx DMA),
            # first group's first, so gather 0 isn't gated on the whole
            # index array
            idx_sbs = []
            idx_off0 = 0
            for g_i, ngg in enumerate(gs):
                gcols = ngg * 8 * F
                idx_g = cpool.tile([P, gcols], i16, tag=f"idx{g_i}")
                nc.sync.dma_start(
                    out=idx_g[:],
                    in_=idxs[:, idx_off0 : idx_off0 + gcols],
                )
                idx_sbs.append(idx_g)
                idx_off0 += gcols
            nc.sync.dma_start(out=dense_sb[:], in_=dense_d[:])
            nc.sync.dma_start(out=wvec_sb[:], in_=wvec_d[:])

            # warmup: one tiny gather per queue, no data deps beyond a
            # memset idx — absorbs the first-gather DGE warmup and the
            # startup semaphore serialization before the real gathers
            idxw = cpool.tile([P, 8], i16)
            gwarm = cpool.tile([P, N_QUEUES, ROWB], f8)
            nc.vector.memset(idxw[:], 0)
            for q in range(N_QUEUES):
                nc.gpsimd.dma_gather(
                    gwarm[:, q : q + 1, :],
                    t2[0:v, :],
                    idxw[:],
                    P,
                    P,
                    ROWB,
                    single_packet=SINGLE_PACKET,
                    queue_num=q,
                )

            gather_seq = 0
            tile_base = 0
            for g_i, ngg in enumerate(gs):
                # gather output must be contiguous, so gall gets a tag per
                # group size; prod below is NGMAX-shaped and sliced (DVE APs
                # handle the stride). The ngg=2 groups are first and last —
                # far apart — so one buffer suffices for them; the ngg=4
                # groups triple-buffer so a group's gathers never stall on
                # compute two groups back.
                gall = gpool.tile([P, F, ngg, ROWB], f8, tag=f"gall{ngg}")
                idx_g = idx_sbs[g_i]
                # reversed window order: the descending-i compute consumes
                # windows last-to-first, so emitting w6 first lets compute
                # start after the first gather of the group lands. idx
                # columns are laid out in emission order within the group.
                idx_off = 0
                for wi, (c0, ncw) in reversed(list(enumerate(WINDOWS))):
                    nidx = ncw * ngg * P
                    ncols = ncw * 8 * ngg
                    nc.gpsimd.dma_gather(
                        gall[:, c0 : c0 + ncw, :, :].rearrange(
                            "p c n r -> p (c n) r"
                        ),
                        t2[c0 * v : (c0 + ncw) * v, :],
                        idx_g[:, idx_off : idx_off + ncols],
                        nidx,
                        nidx,
                        ROWB,
                        single_packet=SINGLE_PACKET,
                        queue_num=gather_seq % N_QUEUES,
                    )
                    gather_seq += 1
                    idx_off += ncols

                prod_f = spool.tile([P, NGMAX, PRODW], bf16, tag="prod")
                prod = prod_f[:, :ngg, :]

                # pair products for all of the group's tiles at once:
                # per smaller-field i, out[p, n, j, d] =
                #   row_i[block j] * row_j[block i]   (j > i)
                off = 0
                for i in reversed(range(F - 1)):
                    cnt = F - 1 - i
                    x = gall[:, i, :, i * D : EMB].rearrange(
                        "p n (c d) -> p n c d", d=D
                    )
                    y = gall[:, i + 1 : F, :, i * D : (i + 1) * D].rearrange(
                        "p c n d -> p n c d"
                    )
                    nc.vector.tensor_tensor(
                        out=prod[:, :, off : off + cnt * D].rearrange(
                            "p n (c d) -> p n c d", d=D
                        ),
                        in0=x,
                        in1=y,
                        op=mult,
                    )
                    off += cnt * D

                # w_sparse values (bf16, pre-scaled by 1024) behind the
                # products so one reduction covers cross + linear_sparse.
                # scalar_tensor_tensor, NOT tensor_scalar: single-src DVE ops
                # enter 2-port perf mode, which takes the SBUF port pair as an
                # exclusive lock and starves SWDGE descriptor generation
                # (gathers stall); two-tensor ops never contend.
                wsp = gall[:, :, :, WSPB : WSPB + 2].bitcast(bf16).rearrange(
                    "p c n one -> p n (c one)"
                )
                nc.vector.scalar_tensor_tensor(
                    out=prod[:, :, NPAIR_ELEMS:NRED],
                    in0=wsp,
                    scalar=1.0,
                    in1=wsp,
                    op0=mult,
                    op1=mybir.AluOpType.bypass,
                )

                last_group = g_i == len(gs) - 1
                accg = None
                if last_group and ngg > 1:
                    # tail trim: odd tiles reduce on DVE (tensor_reduce never
                    # enters 2-port mode, so it can't starve SWDGE) in
                    # parallel with the even tiles' serial ACT accumulates
                    accg = apool.tile([P, NGMAX], f32, tag="accg")
                    nc.vector.tensor_reduce(
                        out=accg[:, 1:ngg:2],
                        in_=prod[:, 1:ngg:2, :NRED],
                        axis=mybir.AxisListType.X,
                        op=mybir.AluOpType.add,
                    )

                for n in range(ngg):
                    tt = tile_base + n
                    lin = apool.tile([P, 1], f32, tag="lin")

                    if accg is not None and n % 2 == 1:
                        acc = accg[:, n : n + 1]
                    else:
                        acc_t = apool.tile([P, 1], f32, tag="acc")
                        acc = acc_t[:]
                        # cross sum + w_sparse sum: one ACT accumulate
                        # (in-place copy; ACT streams read-then-write)
                        nc.scalar.activation(
                            out=prod[:, n, :NRED],
                            in_=prod[:, n, :NRED],
                            func=copy_f,
                            accum_out=acc,
                        )

                    # linear dense + bias on the (idle) tensor engine
                    ps = pspool.tile([P, 1], f32)
                    nc.tensor.matmul(
                        out=ps[:],
                        lhsT=dense_sb[:, tt, :],
                        rhs=wvec_sb[:, :1],
                        start=True,
                        stop=True,
                    )
                    nc.scalar.copy(out=lin[:], in_=ps[:])

                    # sigmoid(acc/1024 + linear)
                    nc.scalar.activation(
                        out=out_all[:, tt : tt + 1],
                        in_=acc,
                        func=sigm_f,
                        scale=1.0 / (SCALE * SCALE),
                        bias=lin[:],
                    )
                tile_base += ngg

            nc.sync.dma_start(out=out[:], in_=out_all[:])

    nc.compile()
    return nc


_PROGRAM_CACHE = {}


def _get_program():
    if "nc" not in _PROGRAM_CACHE:
        _PROGRAM_CACHE["nc"] = _build_program()
    return _PROGRAM_CACHE["nc"]


def make_idx_array(sparse_core, n_tiles=N_TILES, v=V):
    """sparse_core: [BPC, F] local ids (< V). Returns [P, idx_cols] i16.

    Column layout mirrors _build_program: groups per GROUPS, windows
    emitted in reversed order, idx element i at [partition i%16 (replicated
    8x down), col i//16]; within a gather i = (c_local * ngg + n) * 128 + p.
    """
    gs = GROUPS
    spc = sparse_core.reshape(P, n_tiles, F)  # [p, tt, c], sample s = p*n_tiles+tt
    cols = []
    tile_base = 0
    for ngg in gs:
        for wi, (c0, ncw) in reversed(list(enumerate(WINDOWS))):
            vals = spc[:, tile_base : tile_base + ngg, c0 : c0 + ncw].transpose(
                2, 1, 0
            ).astype(np.int64)
            vals = vals + (np.arange(ncw, dtype=np.int64) * v)[:, None, None]
            flat = vals.reshape(-1).astype(np.int16)
            m = len(flat) // 16
            cols.append(np.tile(flat.reshape(m, 16).T, (8, 1)))  # [128, m]
        tile_base += ngg
    return np.ascontiguousarray(np.concatenate(cols, axis=1))


def _prep_inputs(dense_input, sparse_input, tables, w_dense, w_sparse, bias):
    import ml_dtypes

    dense_input = np.asarray(dense_input, dtype=np.float32)
    sparse_input = np.asarray(sparse_input)
    tables = np.asarray(tables, dtype=np.float32)
    w_dense = np.asarray(w_dense, dtype=np.float32)
    w_sparse = np.asarray(w_sparse, dtype=np.float32)
    bias = np.asarray(bias, dtype=np.float32)

    # T2[g] = [tables[t, g, :]*32 as e3m4 for t != g//V] ++ [w_sparse[g]*1024
    # as bf16] ++ pad
    t2u8 = np.zeros((VTOT, ROWB), dtype=np.uint8)
    for c in range(F):
        sl = slice(c * V, (c + 1) * V)
        sel = [t for t in range(F) if t != c]
        emb = tables[sel, sl, :].transpose(1, 0, 2).reshape(V, EMB)
        emb8 = np.clip(emb * SCALE, -CLIP, CLIP).astype(ml_dtypes.float8_e3m4)
        t2u8[sl, :EMB] = emb8.view(np.uint8)
        wspv = (w_sparse[sl, 0] * (SCALE * SCALE)).astype(ml_dtypes.bfloat16)
        t2u8[sl, WSPB : WSPB + 2] = wspv[:, None].view(np.uint8)
    t2 = t2u8.view(ml_dtypes.float8_e3m4)

    sparse_i = sparse_input.astype(np.int64).reshape(N_CORES, BPC, F)
    dense_aug = np.concatenate(
        [dense_input, np.ones((B, 1), dtype=np.float32)], axis=1
    ).reshape(N_CORES, P, N_TILES, DD + 1)
    waug = np.concatenate([w_dense[:, 0], bias]).astype(np.float32)
    wvec = waug.reshape(DD + 1, 1)

    in_maps = []
    for k in range(N_CORES):
        in_maps.append(
            {
                "t2": t2,
                "idxs": make_idx_array(sparse_i[k]),
                "dense": np.ascontiguousarray(dense_aug[k].transpose(2, 1, 0)),
                "wvec": wvec,
            }
        )
    return in_maps


def kernel(dense_input, sparse_input, tables, w_dense, w_sparse, bias, _trace=False):
    _, _, _, bass_utils = _import_concourse()

    nc = _get_program()
    in_maps = _prep_inputs(dense_input, sparse_input, tables, w_dense, w_sparse, bias)
    res = bass_utils.run_bass_kernel_spmd(
        nc, in_maps, core_ids=list(range(N_CORES)), trace=_trace
    )
    outs = [res.results[k]["out"].reshape(BPC) for k in range(N_CORES)]
    full = np.concatenate(outs).reshape(B, 1).astype(np.float32)
    if _trace:
        return full, res
    return full


# revision 33
# speedup vs baseline: 1.1985x; 1.1035x over previous
"""FFM layer (field-aware factorization machine) on 8 Trainium2 cores.

Strategy: data-parallel over batch (2048 samples/core). The embedding tables
are re-laid-out on the host into one row per global vocab id g (owned by
exactly one field c = g // V): the 19 *other* fields' embeddings for that id
in fp8 e3m4 (scaled by 32; values are ~N(0, 0.05^2) so 4-bit-mantissa fp8
at this scale keeps max output rel-err ~1e-3..1e-2, well inside the 2e-2
gate), plus the w_sparse value as bf16 (scaled by 32*32 so it sums in the
same accumulator as the pair products), padded to 512 B (two 256 B dma_gather
granules; 33% less gather traffic than the bf16/768 B layout).

The gather uses nc.gpsimd.dma_gather (int16 indices). Indices must fit int16,
so gathers address vocab windows of 3 fields (3*10000 < 32767), with
window-relative indices. Tiles are processed in groups of 2 (one gather per
(window, group)), single_packet=False so each row is its own packet and the
SDMA engines interleave rows across the 4 SWDGE queues (hides HBM read
latency; ~31 ns/row/engine vs ~38 with one big packet per engine).

Compute per group: 19 DVE tensor_tensor multiplies (one per smaller field i,
batched over the group's tiles via 4-D access patterns) write all pair
products into a [P, ngg, 3072] bf16 scratch; one DVE scalar_tensor_tensor
(two-tensor form — single-src DVE ops enter 2-port perf mode whose exclusive
SBUF port lock starves SWDGE descriptor generation) drops the 20 w_sparse
values in behind them. Per tile, one scalar-engine accumulate reduces
products+wsp to a single f32 per sample (the last group splits this between
ACT and a DVE tensor_reduce to shorten the tail), the tensor engine does the
dense linear part, and a final fused activation computes
sigmoid(acc/1024 + linear).
"""

import os
import sys

import numpy as np


def _import_concourse():
    try:
        import concourse  # noqa: F401
    except ImportError:
        for p in ("/opt/trn_rl_repo", "/root/.axon_site/_ro/trn_rl_repo"):
            if os.path.isdir(p) and p not in sys.path:
                sys.path.insert(0, p)
    import concourse.bass as bass  # noqa: F401
    import concourse.mybir as mybir  # noqa: F401
    import concourse.tile as tile  # noqa: F401
    from concourse import bass_utils  # noqa: F401

    return bass, mybir, tile, bass_utils


# Problem constants (hardcoded per contract)
F = 20          # sparse fields
V = 10000       # vocab per field
VTOT = F * V    # 200000
D = 16          # embed dim
B = 16384       # batch
DD = 13         # dense feature count
N_CORES = 8
P = 128         # SBUF partitions

BPC = B // N_CORES          # 2048 samples per core
N_TILES = BPC // P          # 16 tiles of 128 samples
ROWB = 512                  # gather row bytes (= fp8 elements)
EMB = (F - 1) * D           # 304 fp8 payload elements
WSPB = EMB                  # byte offset of the bf16 w_sparse slot
SCALE = 32.0                # host-side fp8 scale; products come out *1024
CLIP = 15.4375              # e3m4 max normal is 15.5 (inf above)
NPAIR_ELEMS = (F * (F - 1) // 2) * D  # 3040 pair-product elements per sample
NRED = NPAIR_ELEMS + F      # +20 w_sparse values reduced in the same pass
PRODW = 3072                # per-tile stride in the product scratch
COLS_PER_WIN = 3            # fields per gather window (3*V < int16 max)
SINGLE_PACKET = False
N_QUEUES = 4
GROUPS = [2] * 8            # tiles per gather group
NGMAX = max(GROUPS)

WINDOWS = [
    (c0, min(COLS_PER_WIN, F - c0)) for c0 in range(0, F, COLS_PER_WIN)
]
NW = len(WINDOWS)


def _patch_queue_lanes():
    """Make Tile assign DMASW sem lanes per SWDGE queue (lane 2q/2q+1 for
    queue q) — the runtime locks each lane to one queue, but stock Tile
    round-robins lanes obliviously."""
    import concourse.tile_sem_assignment as tsa

    if getattr(tsa, "_ffm_queue_patch", False):
        return
    import concourse.mybir as mybir

    orig = tsa.TileClockTick._assign_tick

    def patched(self, inst):
        q = getattr(inst, "queue_num", None)
        if (
            q is not None
            and isinstance(inst, tsa.DMAInst)
            and inst.engine == mybir.EngineType.Pool
        ):
            state = getattr(self, "_ffm_perq", None)
            if state is None:
                state = {}
                self._ffm_perq = state
            self.next_sw_dma_idx = 2 * q + state.get(q, 0)
            orig(self, inst)
            state[q] = state.get(q, 0) ^ 1
            return
        orig(self, inst)

    tsa.TileClockTick._assign_tick = patched
    tsa._ffm_queue_patch = True


def _build_program(n_tiles=N_TILES, vtot=VTOT, for_sim=False):
    bass, mybir, tile, _ = _import_concourse()
    _patch_queue_lanes()

    v = vtot // F
    gs = GROUPS

    import concourse.bacc as bacc

    # Bacc (not plain Bass): its compile() runs generate_event_semaphores,
    # which splits multi-semaphore waits into InstEventSemaphore prefixes —
    # TRN2 instructions can carry only one inline wait — and inserts the
    # GPSIMD ucode library loads that dma_gather needs.
    nc = bacc.Bacc(None, target_bir_lowering=False, debug=for_sim,
                   num_swdge_queues=N_QUEUES)

    f32 = mybir.dt.float32
    bf16 = mybir.dt.bfloat16
    f8 = mybir.dt.float8e3
    i16 = mybir.dt.int16
    mult = mybir.AluOpType.mult
    copy_f = mybir.ActivationFunctionType.Copy
    sigm_f = mybir.ActivationFunctionType.Sigmoid

    t2 = nc.dram_tensor("t2", [vtot, ROWB], f8, kind="ExternalInput")
    idx_cols = sum(ncw * 8 * g for g in gs for (c0, ncw) in WINDOWS)
    idxs = nc.dram_tensor("idxs", [P, idx_cols], i16, kind="ExternalInput")
    dense_d = nc.dram_tensor("dense", [DD + 1, n_tiles, P], f32, kind="ExternalInput")
    wvec_d = nc.dram_tensor("wvec", [DD + 1, 1], f32, kind="ExternalInput")
    out = nc.dram_tensor("out", [P, n_tiles], f32, kind="ExternalOutput")

    with tile.TileContext(nc) as tc:
        with (
            tc.tile_pool(name="const", bufs=1) as cpool,
            tc.tile_pool(name="gather", bufs=3) as gpool,
            tc.tile_pool(name="scratch", bufs=2) as spool,
            tc.tile_pool(name="accp", bufs=4) as apool,
            tc.tile_pool(name="psum", bufs=2, space="PSUM") as pspool,
        ):
            dense_sb = cpool.tile([DD + 1, n_tiles, P], f32)
            wvec_sb = cpool.tile([DD + 1, 1], f32)
            out_all = cpool.tile([P, n_tiles], f32)

            # per-group idx tiles (separate tiles, not slices of one tile —
            # Tile would otherwise make gather 0 wait on every idx DMA),
            # first group's first, so gather 0 isn't gated on the whole
            # index array
            idx_sbs = []
            idx_off0 = 0
            for g_i, ngg in enumerate(gs):
                gcols = ngg * 8 * F
                idx_g = cpool.tile([P, gcols], i16, tag=f"idx{g_i}")
                nc.sync.dma_start(
                    out=idx_g[:],
                    in_=idxs[:, idx_off0 : idx_off0 + gcols],
                )
                idx_sbs.append(idx_g)
                idx_off0 += gcols
            nc.sync.dma_start(out=dense_sb[:], in_=dense_d[:])
            nc.sync.dma_start(out=wvec_sb[:], in_=wvec_d[:])

            # warmup: one tiny gather per queue, no data deps beyond a
            # memset idx — absorbs the first-gather DGE warmup and the
            # startup semaphore serialization before the real gathers
            idxw = cpool.tile([P, 8], i16)
            gwarm = cpool.tile([P, N_QUEUES, ROWB], f8)
            nc.vector.memset(idxw[:], 0)
            for q in range(N_QUEUES):
                nc.gpsimd.dma_gather(
                    gwarm[:, q : q + 1, :],
                    t2[0:v, :],
                    idxw[:],
                    P,
                    P,
                    ROWB,
                    single_packet=SINGLE_PACKET,
                    queue_num=q,
                )

            gather_seq = 0
            tile_base = 0
            for g_i, ngg in enumerate(gs):
                # gather output must be contiguous, so gall is a full tile
                # per group size (not a slice of a shared max-size tile);
                # triple-buffered so a group's gathers never stall on
                # compute two groups back
                gall = gpool.tile([P, F, ngg, ROWB], f8, tag=f"gall{ngg}")
                idx_g = idx_sbs[g_i]
                # reversed window order: the descending-i compute consumes
                # windows last-to-first, so emitting w6 first lets compute
                # start after the first gather of the group lands. idx
                # columns are laid out in emission order within the group.
                idx_off = 0
                for wi, (c0, ncw) in reversed(list(enumerate(WINDOWS))):
                    nidx = ncw * ngg * P
                    ncols = ncw * 8 * ngg
                    nc.gpsimd.dma_gather(
                        gall[:, c0 : c0 + ncw, :, :].rearrange(
                            "p c n r -> p (c n) r"
                        ),
                        t2[c0 * v : (c0 + ncw) * v, :],
                        idx_g[:, idx_off : idx_off + ncols],
                        nidx,
                        nidx,
                        ROWB,
                        single_packet=SINGLE_PACKET,
                        queue_num=gather_seq % N_QUEUES,
                    )
                    gather_seq += 1
                    idx_off += ncols

                prod_f = spool.tile([P, NGMAX, PRODW], bf16, tag="prod")
                prod = prod_f[:, :ngg, :]

                # pair products for all of the group's tiles at once:
                # per smaller-field i, out[p, n, j, d] =
                #   row_i[block j] * row_j[block i]   (j > i)
                off = 0
                for i in reversed(range(F - 1)):
                    cnt = F - 1 - i
                    x = gall[:, i, :, i * D : EMB].rearrange(
                        "p n (c d) -> p n c d", d=D
                    )
                    y = gall[:, i + 1 : F, :, i * D : (i + 1) * D].rearrange(
                        "p c n d -> p n c d"
                    )
                    nc.vector.tensor_tensor(
                        out=prod[:, :, off : off + cnt * D].rearrange(
                            "p n (c d) -> p n c d", d=D
                        ),
                        in0=x,
                        in1=y,
                        op=mult,
                    )
                    off += cnt * D

                # w_sparse values (bf16, pre-scaled by 1024) behind the
                # products so one reduction covers cross + linear_sparse.
                # scalar_tensor_tensor, NOT tensor_scalar: single-src DVE ops
                # enter 2-port perf mode, which takes the SBUF port pair as an
                # exclusive lock and starves SWDGE descriptor generation
                # (gathers stall); two-tensor ops never contend.
                wsp = gall[:, :, :, WSPB : WSPB + 2].bitcast(bf16).rearrange(
                    "p c n one -> p n (c one)"
                )
                nc.vector.scalar_tensor_tensor(
                    out=prod[:, :, NPAIR_ELEMS:NRED],
                    in0=wsp,
                    scalar=1.0,
                    in1=wsp,
                    op0=mult,
                    op1=mybir.AluOpType.bypass,
                )

                last_group = g_i == len(gs) - 1
                accg = None
                if last_group and ngg > 1:
                    # tail trim: odd tiles reduce on DVE (tensor_reduce never
                    # enters 2-port mode, so it can't starve SWDGE) in
                    # parallel with the even tiles' serial ACT accumulates
                    accg = apool.tile([P, NGMAX], f32, tag="accg")
                    nc.vector.tensor_reduce(
                        out=accg[:, 1:ngg:2],
                        in_=prod[:, 1:ngg:2, :NRED],
                        axis=mybir.AxisListType.X,
                        op=mybir.AluOpType.add,
                    )

                for n in range(ngg):
                    tt = tile_base + n
                    lin = apool.tile([P, 1], f32, tag="lin")

                    if accg is not None and n % 2 == 1:
                        acc = accg[:, n : n + 1]
                    else:
                        acc_t = apool.tile([P, 1], f32, tag="acc")
                        acc = acc_t[:]
                        # cross sum + w_sparse sum: one ACT accumulate
                        # (in-place copy; ACT streams read-then-write)
                        nc.scalar.activation(
                            out=prod[:, n, :NRED],
                            in_=prod[:, n, :NRED],
                            func=copy_f,
                            accum_out=acc,
                        )

                    # linear dense + bias on the (idle) tensor engine
                    ps = pspool.tile([P, 1], f32)
                    nc.tensor.matmul(
                        out=ps[:],
                        lhsT=dense_sb[:, tt, :],
                        rhs=wvec_sb[:, :1],
                        start=True,
                        stop=True,
                    )
                    nc.scalar.copy(out=lin[:], in_=ps[:])

                    # sigmoid(acc/1024 + linear)
                    nc.scalar.activation(
                        out=out_all[:, tt : tt + 1],
                        in_=acc,
                        func=sigm_f,
                        scale=1.0 / (SCALE * SCALE),
                        bias=lin[:],
                    )
                tile_base += ngg

            nc.sync.dma_start(out=out[:], in_=out_all[:])

    nc.compile()
    return nc


_PROGRAM_CACHE = {}


def _get_program():
    if "nc" not in _PROGRAM_CACHE:
        _PROGRAM_CACHE["nc"] = _build_program()
    return _PROGRAM_CACHE["nc"]


def make_idx_array(sparse_core, n_tiles=N_TILES, v=V):
    """sparse_core: [BPC, F] local ids (< V). Returns [P, idx_cols] i16.

    Column layout mirrors _build_program: groups per GROUPS, windows
    emitted in reversed order, idx element i at [partition i%16 (replicated
    8x down), col i//16]; within a gather i = (c_local * ngg + n) * 128 + p.
    """
    gs = GROUPS
    spc = sparse_core.reshape(P, n_tiles, F)  # [p, tt, c], sample s = p*n_tiles+tt
    cols = []
    tile_base = 0
    for ngg in gs:
        for wi, (c0, ncw) in reversed(list(enumerate(WINDOWS))):
            vals = spc[:, tile_base : tile_base + ngg, c0 : c0 + ncw].transpose(
                2, 1, 0
            ).astype(np.int64)
            vals = vals + (np.arange(ncw, dtype=np.int64) * v)[:, None, None]
            flat = vals.reshape(-1).astype(np.int16)
            m = len(flat) // 16
            cols.append(np.tile(flat.reshape(m, 16).T, (8, 1)))  # [128, m]
        tile_base += ngg
    return np.ascontiguousarray(np.concatenate(cols, axis=1))


def _prep_inputs(dense_input, sparse_input, tables, w_dense, w_sparse, bias):
    import ml_dtypes

    dense_input = np.asarray(dense_input, dtype=np.float32)
    sparse_input = np.asarray(sparse_input)
    tables = np.asarray(tables, dtype=np.float32)
    w_dense = np.asarray(w_dense, dtype=np.float32)
    w_sparse = np.asarray(w_sparse, dtype=np.float32)
    bias = np.asarray(bias, dtype=np.float32)

    # T2[g] = [tables[t, g, :]*32 as e3m4 for t != g//V] ++ [w_sparse[g]*1024
    # as bf16] ++ pad
    t2u8 = np.zeros((VTOT, ROWB), dtype=np.uint8)
    for c in range(F):
        sl = slice(c * V, (c + 1) * V)
        sel = [t for t in range(F) if t != c]
        emb = tables[sel, sl, :].transpose(1, 0, 2).reshape(V, EMB)
        emb8 = np.clip(emb * SCALE, -CLIP, CLIP).astype(ml_dtypes.float8_e3m4)
        t2u8[sl, :EMB] = emb8.view(np.uint8)
        wspv = (w_sparse[sl, 0] * (SCALE * SCALE)).astype(ml_dtypes.bfloat16)
        t2u8[sl, WSPB : WSPB + 2] = wspv[:, None].view(np.uint8)
    t2 = t2u8.view(ml_dtypes.float8_e3m4)

    sparse_i = sparse_input.astype(np.int64).reshape(N_CORES, BPC, F)
    dense_aug = np.concatenate(
        [dense_input, np.ones((B, 1), dtype=np.float32)], axis=1
    ).reshape(N_CORES, P, N_TILES, DD + 1)
    waug = np.concatenate([w_dense[:, 0], bias]).astype(np.float32)
    wvec = waug.reshape(DD + 1, 1)

    in_maps = []
    for k in range(N_CORES):
        in_maps.append(
            {
                "t2": t2,
                "idxs": make_idx_array(sparse_i[k]),
                "dense": np.ascontiguousarray(dense_aug[k].transpose(2, 1, 0)),
                "wvec": wvec,
            }
        )
    return in_maps


def kernel(dense_input, sparse_input, tables, w_dense, w_sparse, bias, _trace=False):
    _, _, _, bass_utils = _import_concourse()

    nc = _get_program()
    in_maps = _prep_inputs(dense_input, sparse_input, tables, w_dense, w_sparse, bias)
    res = bass_utils.run_bass_kernel_spmd(
        nc, in_maps, core_ids=list(range(N_CORES)), trace=_trace
    )
    outs = [res.results[k]["out"].reshape(BPC) for k in range(N_CORES)]
    full = np.concatenate(outs).reshape(B, 1).astype(np.float32)
    if _trace:
        return full, res
    return full


# revision 39
# speedup vs baseline: 1.2001x; 1.0013x over previous
"""FFM layer (field-aware factorization machine) on 8 Trainium2 cores.

Strategy: data-parallel over batch (2048 samples/core). The embedding tables
are re-laid-out on the host into one row per global vocab id g (owned by
exactly one field c = g // V): the 19 *other* fields' embeddings for that id
in fp8 e3m4 (scaled by 32; values are ~N(0, 0.05^2) so 4-bit-mantissa fp8
at this scale keeps max output rel-err ~1e-3..1e-2, well inside the 2e-2
gate), plus the w_sparse value as bf16 (scaled by 32*32 so it sums in the
same accumulator as the pair products), padded to 512 B (two 256 B dma_gather
granules; 33% less gather traffic than the bf16/768 B layout).

The gather uses nc.gpsimd.dma_gather (int16 indices). Indices must fit int16,
so gathers address vocab windows of 3 fields (3*10000 < 32767), with
window-relative indices. Tiles are processed in groups of 2 (one gather per
(window, group)), single_packet=False so each row is its own packet and the
SDMA engines interleave rows across the 4 SWDGE queues (hides HBM read
latency; ~31 ns/row/engine vs ~38 with one big packet per engine).

Compute per group: 19 DVE tensor_tensor multiplies (one per smaller field i,
batched over the group's tiles via 4-D access patterns) write all pair
products into a [P, ngg, 3072] bf16 scratch; one DVE scalar_tensor_tensor
(two-tensor form — single-src DVE ops enter 2-port perf mode whose exclusive
SBUF port lock starves SWDGE descriptor generation) drops the 20 w_sparse
values in behind them. Per tile, one scalar-engine accumulate reduces
products+wsp to a single f32 per sample (the last group splits this between
ACT and a DVE tensor_reduce to shorten the tail), the tensor engine does the
dense linear part, and a final fused activation computes
sigmoid(acc/1024 + linear).
"""

import os
import sys

import numpy as np


def _import_concourse():
    try:
        import concourse  # noqa: F401
    except ImportError:
        for p in ("/opt/trn_rl_repo", "/root/.axon_site/_ro/trn_rl_repo"):
            if os.path.isdir(p) and p not in sys.path:
                sys.path.insert(0, p)
    import concourse.bass as bass  # noqa: F401
    import concourse.mybir as mybir  # noqa: F401
    import concourse.tile as tile  # noqa: F401
    from concourse import bass_utils  # noqa: F401

    return bass, mybir, tile, bass_utils


# Problem constants (hardcoded per contract)
F = 20          # sparse fields
V = 10000       # vocab per field
VTOT = F * V    # 200000
D = 16          # embed dim
B = 16384       # batch
DD = 13         # dense feature count
N_CORES = 8
P = 128         # SBUF partitions

BPC = B // N_CORES          # 2048 samples per core
N_TILES = BPC // P          # 16 tiles of 128 samples
ROWSTRIDE = 512             # row stride in the HBM table (must be /256)
ROWB = 384                  # gathered bytes per row (payload 306 B)
EMB = (F - 1) * D           # 304 fp8 payload elements
WSPB = EMB                  # byte offset of the bf16 w_sparse slot
SCALE = 32.0                # host-side fp8 scale; products come out *1024
CLIP = 15.4375              # e3m4 max normal is 15.5 (inf above)
NPAIR_ELEMS = (F * (F - 1) // 2) * D  # 3040 pair-product elements per sample
NRED = NPAIR_ELEMS + F      # +20 w_sparse values reduced in the same pass
PRODW = 3072                # per-tile stride in the product scratch
COLS_PER_WIN = 3            # fields per gather window (3*V < int16 max)
SINGLE_PACKET = False
N_QUEUES = 4
GROUPS = [2] * 8            # tiles per gather group
NGMAX = max(GROUPS)

WINDOWS = [
    (c0, min(COLS_PER_WIN, F - c0)) for c0 in range(0, F, COLS_PER_WIN)
]
NW = len(WINDOWS)


def _patch_queue_lanes():
    """Make Tile assign DMASW sem lanes per SWDGE queue (lane 2q/2q+1 for
    queue q) — the runtime locks each lane to one queue, but stock Tile
    round-robins lanes obliviously."""
    import concourse.tile_sem_assignment as tsa

    if getattr(tsa, "_ffm_queue_patch", False):
        return
    import concourse.mybir as mybir

    orig = tsa.TileClockTick._assign_tick

    def patched(self, inst):
        q = getattr(inst, "queue_num", None)
        if (
            q is not None
            and isinstance(inst, tsa.DMAInst)
            and inst.engine == mybir.EngineType.Pool
        ):
            state = getattr(self, "_ffm_perq", None)
            if state is None:
                state = {}
                self._ffm_perq = state
            self.next_sw_dma_idx = 2 * q + state.get(q, 0)
            orig(self, inst)
            state[q] = state.get(q, 0) ^ 1
            return
        orig(self, inst)

    tsa.TileClockTick._assign_tick = patched
    tsa._ffm_queue_patch = True


def _dma_gather_raw(gp, out_ap, in_ap, idxs_ap, num_idxs, elem_size_bytes,
                    elem_step_bytes, single_packet, queue_num):
    """dma_gather for non-256-multiple elem_size (bass asserts %256==0 as a
    'transpose restriction', but the non-transpose descriptor path moves
    arbitrary byte counts; the row STRIDE still must be a 256 multiple).
    Mirrors BassGpSimd.dma_gather's non-transpose DRAM-source lowering."""
    import concourse.mybir as mybir

    gp._assert_queue_num(queue_num)
    assert idxs_ap.dtype == mybir.dt.int16
    assert in_ap.dtype == out_ap.dtype
    assert elem_step_bytes % 256 == 0
    inst = gp.add_instruction(
        mybir.InstDMAGatherAnt(
            name=gp.bass.get_next_instruction_name(),
            ins=[
                *gp.lower_ap_dma(in_ap, for_custom_bir_dma=True),
                gp.lower_ap(idxs_ap),
                gp.lower_val_access(gp.to_reg(num_idxs)),
            ],
            outs=[gp.lower_ap(out_ap)],
            transpose=False,
            num_idxs=num_idxs,
            elem_size=elem_size_bytes,
            stride_bytes_256=elem_step_bytes // 256,
            gen_mode=0,
            single_packet=single_packet,
            queue_num=queue_num,
            sbuf_tokens_per_rank=0,
            sbuf_free_dim_per_rank=0,
            sbuf_free_dim_pad_per_rank=0,
            sbuf_byte_offset=0,
        )
    )
    return inst


def _build_program(n_tiles=N_TILES, vtot=VTOT, for_sim=False):
    bass, mybir, tile, _ = _import_concourse()
    _patch_queue_lanes()

    v = vtot // F
    gs = GROUPS

    import concourse.bacc as bacc

    # Bacc (not plain Bass): its compile() runs generate_event_semaphores,
    # which splits multi-semaphore waits into InstEventSemaphore prefixes —
    # TRN2 instructions can carry only one inline wait — and inserts the
    # GPSIMD ucode library loads that dma_gather needs.
    nc = bacc.Bacc(None, target_bir_lowering=False, debug=for_sim,
                   num_swdge_queues=N_QUEUES)

    f32 = mybir.dt.float32
    bf16 = mybir.dt.bfloat16
    f8 = mybir.dt.float8e3
    i16 = mybir.dt.int16
    mult = mybir.AluOpType.mult
    copy_f = mybir.ActivationFunctionType.Copy
    sigm_f = mybir.ActivationFunctionType.Sigmoid

    t2 = nc.dram_tensor("t2", [vtot, ROWSTRIDE], f8, kind="ExternalInput")
    idx_cols = sum(ncw * 8 * g for g in gs for (c0, ncw) in WINDOWS)
    idxs = nc.dram_tensor("idxs", [P, idx_cols], i16, kind="ExternalInput")
    dense_d = nc.dram_tensor("dense", [DD + 1, n_tiles, P], f32, kind="ExternalInput")
    wvec_d = nc.dram_tensor("wvec", [DD + 1, 1], f32, kind="ExternalInput")
    out = nc.dram_tensor("out", [P, n_tiles], f32, kind="ExternalOutput")

    with tile.TileContext(nc) as tc:
        with (
            tc.tile_pool(name="const", bufs=1) as cpool,
            tc.tile_pool(name="gather", bufs=3) as gpool,
            tc.tile_pool(name="scratch", bufs=2) as spool,
            tc.tile_pool(name="accp", bufs=4) as apool,
            tc.tile_pool(name="psum", bufs=2, space="PSUM") as pspool,
        ):
            dense_sb = cpool.tile([DD + 1, n_tiles, P], f32)
            wvec_sb = cpool.tile([DD + 1, 1], f32)
            out_all = cpool.tile([P, n_tiles], f32)

            # per-group idx tiles (separate tiles, not slices of one tile —
            # Tile would otherwise make gather 0 wait on every idx DMA),
            # first group's first, so gather 0 isn't gated on the whole
            # index array
            idx_sbs = []
            idx_off0 = 0
            for g_i, ngg in enumerate(gs):
                gcols = ngg * 8 * F
                idx_g = cpool.tile([P, gcols], i16, tag=f"idx{g_i}")
                nc.sync.dma_start(
                    out=idx_g[:],
                    in_=idxs[:, idx_off0 : idx_off0 + gcols],
                )
                idx_sbs.append(idx_g)
                idx_off0 += gcols
            nc.sync.dma_start(out=dense_sb[:], in_=dense_d[:])
            nc.sync.dma_start(out=wvec_sb[:], in_=wvec_d[:])

            # warmup: one tiny gather per queue, no data deps beyond a
            # memset idx — absorbs the first-gather DGE warmup and the
            # startup semaphore serialization before the real gathers
            idxw = cpool.tile([P, 8], i16)
            gwarm = cpool.tile([P, N_QUEUES, ROWB], f8)
            nc.vector.memset(idxw[:], 0)
            for q in range(N_QUEUES):
                _dma_gather_raw(
                    nc.gpsimd,
                    gwarm[:, q : q + 1, :],
                    t2[0:v, :ROWB],
                    idxw[:],
                    P,
                    ROWB,
                    ROWSTRIDE,
                    SINGLE_PACKET,
                    q,
                )

            gather_seq = 0
            tile_base = 0
            for g_i, ngg in enumerate(gs):
                # gather output must be contiguous, so gall is a full tile
                # per group size (not a slice of a shared max-size tile);
                # triple-buffered so a group's gathers never stall on
                # compute two groups back
                gall = gpool.tile([P, F, ngg, ROWB], f8, tag=f"gall{ngg}")
                idx_g = idx_sbs[g_i]
                # reversed window order: the descending-i compute consumes
                # windows last-to-first, so emitting w6 first lets compute
                # start after the first gather of the group lands. idx
                # columns are laid out in emission order within the group.
                idx_off = 0
                for wi, (c0, ncw) in reversed(list(enumerate(WINDOWS))):
                    nidx = ncw * ngg * P
                    ncols = ncw * 8 * ngg
                    _dma_gather_raw(
                        nc.gpsimd,
                        gall[:, c0 : c0 + ncw, :, :].rearrange(
                            "p c n r -> p (c n) r"
                        ),
                        t2[c0 * v : (c0 + ncw) * v, :ROWB],
                        idx_g[:, idx_off : idx_off + ncols],
                        nidx,
                        ROWB,
                        ROWSTRIDE,
                        SINGLE_PACKET,
                        gather_seq % N_QUEUES,
                    )
                    gather_seq += 1
                    idx_off += ncols

                prod_f = spool.tile([P, NGMAX, PRODW], bf16, tag="prod")
                prod = prod_f[:, :ngg, :]

                # pair products for all of the group's tiles at once:
                # per smaller-field i, out[p, n, j, d] =
                #   row_i[block j] * row_j[block i]   (j > i)
                off = 0
                for i in reversed(range(F - 1)):
                    cnt = F - 1 - i
                    x = gall[:, i, :, i * D : EMB].rearrange(
                        "p n (c d) -> p n c d", d=D
                    )
                    y = gall[:, i + 1 : F, :, i * D : (i + 1) * D].rearrange(
                        "p c n d -> p n c d"
                    )
                    nc.vector.tensor_tensor(
                        out=prod[:, :, off : off + cnt * D].rearrange(
                            "p n (c d) -> p n c d", d=D
                        ),
                        in0=x,
                        in1=y,
                        op=mult,
                    )
                    off += cnt * D

                # w_sparse values (bf16, pre-scaled by 1024) behind the
                # products so one reduction covers cross + linear_sparse.
                # scalar_tensor_tensor, NOT tensor_scalar: single-src DVE ops
                # enter 2-port perf mode, which takes the SBUF port pair as an
                # exclusive lock and starves SWDGE descriptor generation
                # (gathers stall); two-tensor ops never contend.
                wsp = gall[:, :, :, WSPB : WSPB + 2].bitcast(bf16).rearrange(
                    "p c n one -> p n (c one)"
                )
                nc.vector.scalar_tensor_tensor(
                    out=prod[:, :, NPAIR_ELEMS:NRED],
                    in0=wsp,
                    scalar=1.0,
                    in1=wsp,
                    op0=mult,
                    op1=mybir.AluOpType.bypass,
                )

                last_group = g_i == len(gs) - 1
                accg = None
                if last_group and ngg > 1:
                    # tail trim: odd tiles reduce on DVE (tensor_reduce never
                    # enters 2-port mode, so it can't starve SWDGE) in
                    # parallel with the even tiles' serial ACT accumulates
                    accg = apool.tile([P, NGMAX], f32, tag="accg")
                    nc.vector.tensor_reduce(
                        out=accg[:, 1:ngg:2],
                        in_=prod[:, 1:ngg:2, :NRED],
                        axis=mybir.AxisListType.X,
                        op=mybir.AluOpType.add,
                    )

                for n in range(ngg):
                    tt = tile_base + n
                    lin = apool.tile([P, 1], f32, tag="lin")

                    if accg is not None and n % 2 == 1:
                        acc = accg[:, n : n + 1]
                    else:
                        acc_t = apool.tile([P, 1], f32, tag="acc")
                        acc = acc_t[:]
                        # cross sum + w_sparse sum: one ACT accumulate
                        # (in-place copy; ACT streams read-then-write)
                        nc.scalar.activation(
                            out=prod[:, n, :NRED],
                            in_=prod[:, n, :NRED],
                            func=copy_f,
                            accum_out=acc,
                        )

                    # linear dense + bias on the (idle) tensor engine
                    ps = pspool.tile([P, 1], f32)
                    nc.tensor.matmul(
                        out=ps[:],
                        lhsT=dense_sb[:, tt, :],
                        rhs=wvec_sb[:, :1],
                        start=True,
                        stop=True,
                    )
                    nc.scalar.copy(out=lin[:], in_=ps[:])

                    # sigmoid(acc/1024 + linear)
                    nc.scalar.activation(
                        out=out_all[:, tt : tt + 1],
                        in_=acc,
                        func=sigm_f,
                        scale=1.0 / (SCALE * SCALE),
                        bias=lin[:],
                    )
                tile_base += ngg

            nc.sync.dma_start(out=out[:], in_=out_all[:])

    nc.compile()
    return nc


_PROGRAM_CACHE = {}


def _get_program():
    if "nc" not in _PROGRAM_CACHE:
        _PROGRAM_CACHE["nc"] = _build_program()
    return _PROGRAM_CACHE["nc"]


def make_idx_array(sparse_core, n_tiles=N_TILES, v=V):
    """sparse_core: [BPC, F] local ids (< V). Returns [P, idx_cols] i16.

    Column layout mirrors _build_program: groups per GROUPS, windows
    emitted in reversed order, idx element i at [partition i%16 (replicated
    8x down), col i//16]; within a gather i = (c_local * ngg + n) * 128 + p.
    """
    gs = GROUPS
    spc = sparse_core.reshape(P, n_tiles, F)  # [p, tt, c], sample s = p*n_tiles+tt
    cols = []
    tile_base = 0
    for ngg in gs:
        for wi, (c0, ncw) in reversed(list(enumerate(WINDOWS))):
            vals = spc[:, tile_base : tile_base + ngg, c0 : c0 + ncw].transpose(
                2, 1, 0
            ).astype(np.int64)
            vals = vals + (np.arange(ncw, dtype=np.int64) * v)[:, None, None]
            flat = vals.reshape(-1).astype(np.int16)
            m = len(flat) // 16
            cols.append(np.tile(flat.reshape(m, 16).T, (8, 1)))  # [128, m]
        tile_base += ngg
    return np.ascontiguousarray(np.concatenate(cols, axis=1))


def _prep_inputs(dense_input, sparse_input, tables, w_dense, w_sparse, bias):
    import ml_dtypes

    dense_input = np.asarray(dense_input, dtype=np.float32)
    sparse_input = np.asarray(sparse_input)
    tables = np.asarray(tables, dtype=np.float32)
    w_dense = np.asarray(w_dense, dtype=np.float32)
    w_sparse = np.asarray(w_sparse, dtype=np.float32)
    bias = np.asarray(bias, dtype=np.float32)

    # T2[g] = [tables[t, g, :]*32 as e3m4 for t != g//V] ++ [w_sparse[g]*1024
    # as bf16] ++ pad
    t2u8 = np.zeros((VTOT, ROWSTRIDE), dtype=np.uint8)
    for c in range(F):
        sl = slice(c * V, (c + 1) * V)
        sel = [t for t in range(F) if t != c]
        emb = tables[sel, sl, :].transpose(1, 0, 2).reshape(V, EMB)
        emb8 = np.clip(emb * SCALE, -CLIP, CLIP).astype(ml_dtypes.float8_e3m4)
        t2u8[sl, :EMB] = emb8.view(np.uint8)
        wspv = (w_sparse[sl, 0] * (SCALE * SCALE)).astype(ml_dtypes.bfloat16)
        t2u8[sl, WSPB : WSPB + 2] = wspv[:, None].view(np.uint8)
    t2 = t2u8.view(ml_dtypes.float8_e3m4)

    sparse_i = sparse_input.astype(np.int64).reshape(N_CORES, BPC, F)
    dense_aug = np.concatenate(
        [dense_input, np.ones((B, 1), dtype=np.float32)], axis=1
    ).reshape(N_CORES, P, N_TILES, DD + 1)
    waug = np.concatenate([w_dense[:, 0], bias]).astype(np.float32)
    wvec = waug.reshape(DD + 1, 1)

    in_maps = []
    for k in range(N_CORES):
        in_maps.append(
            {
                "t2": t2,
                "idxs": make_idx_array(sparse_i[k]),
                "dense": np.ascontiguousarray(dense_aug[k].transpose(2, 1, 0)),
                "wvec": wvec,
            }
        )
    return in_maps


def kernel(dense_input, sparse_input, tables, w_dense, w_sparse, bias, _trace=False):
    _, _, _, bass_utils = _import_concourse()

    nc = _get_program()
    in_maps = _prep_inputs(dense_input, sparse_input, tables, w_dense, w_sparse, bias)
    res = bass_utils.run_bass_kernel_spmd(
        nc, in_maps, core_ids=list(range(N_CORES)), trace=_trace
    )
    outs = [res.results[k]["out"].reshape(BPC) for k in range(N_CORES)]
    full = np.concatenate(outs).reshape(B, 1).astype(np.float32)
    if _trace:
        return full, res
    return full


# revision 40
# speedup vs baseline: 1.2215x; 1.0178x over previous
"""FFM layer (field-aware factorization machine) on 8 Trainium2 cores.

Strategy: data-parallel over batch (2048 samples/core). The embedding tables
are re-laid-out on the host into one row per global vocab id g (owned by
exactly one field c = g // V): the 19 *other* fields' embeddings for that id
in fp8 e3m4 (scaled by 32; values are ~N(0, 0.05^2) so 4-bit-mantissa fp8
at this scale keeps max output rel-err ~1e-3..1e-2, well inside the 2e-2
gate), plus the w_sparse value as bf16 (scaled by 32*32 so it sums in the
same accumulator as the pair products), padded to 512 B (two 256 B dma_gather
granules; 33% less gather traffic than the bf16/768 B layout).

The gather uses nc.gpsimd.dma_gather (int16 indices). Indices must fit int16,
so gathers address vocab windows of 3 fields (3*10000 < 32767), with
window-relative indices. Tiles are processed in groups of 2 (one gather per
(window, group)), single_packet=False so each row is its own packet and the
SDMA engines interleave rows across the 4 SWDGE queues (hides HBM read
latency; ~31 ns/row/engine vs ~38 with one big packet per engine).

Compute per group: 19 DVE tensor_tensor multiplies (one per smaller field i,
batched over the group's tiles via 4-D access patterns) write all pair
products into a [P, ngg, 3072] bf16 scratch; one DVE scalar_tensor_tensor
(two-tensor form — single-src DVE ops enter 2-port perf mode whose exclusive
SBUF port lock starves SWDGE descriptor generation) drops the 20 w_sparse
values in behind them. Per tile, one scalar-engine accumulate reduces
products+wsp to a single f32 per sample (the last group splits this between
ACT and a DVE tensor_reduce to shorten the tail), the tensor engine does the
dense linear part, and a final fused activation computes
sigmoid(acc/1024 + linear).
"""

import os
import sys

import numpy as np


def _import_concourse():
    try:
        import concourse  # noqa: F401
    except ImportError:
        for p in ("/opt/trn_rl_repo", "/root/.axon_site/_ro/trn_rl_repo"):
            if os.path.isdir(p) and p not in sys.path:
                sys.path.insert(0, p)
    import concourse.bass as bass  # noqa: F401
    import concourse.mybir as mybir  # noqa: F401
    import concourse.tile as tile  # noqa: F401
    from concourse import bass_utils  # noqa: F401

    return bass, mybir, tile, bass_utils


# Problem constants (hardcoded per contract)
F = 20          # sparse fields
V = 10000       # vocab per field
VTOT = F * V    # 200000
D = 16          # embed dim
B = 16384       # batch
DD = 13         # dense feature count
N_CORES = 8
P = 128         # SBUF partitions

BPC = B // N_CORES          # 2048 samples per core
N_TILES = BPC // P          # 16 tiles of 128 samples
ROWSTRIDE = 512             # row stride in the HBM table (must be /256)
ROWB = 512                  # gathered bytes per row (payload 306 B; tested
                            # 384 B: 34.1 ns/row vs 31.4 — sub-512 B
                            # descriptors pay the SDMA RMW penalty)
EMB = (F - 1) * D           # 304 fp8 payload elements
WSPB = EMB                  # byte offset of the bf16 w_sparse slot
SCALE = 32.0                # host-side fp8 scale; products come out *1024
CLIP = 15.4375              # e3m4 max normal is 15.5 (inf above)
NPAIR_ELEMS = (F * (F - 1) // 2) * D  # 3040 pair-product elements per sample
NRED = NPAIR_ELEMS + F      # +20 w_sparse values reduced in the same pass
PRODW = 3072                # per-tile stride in the product scratch
COLS_PER_WIN = 3            # fields per gather window (3*V < int16 max)
SINGLE_PACKET = False
N_QUEUES = 4
GROUPS = [2] * 8            # tiles per gather group
NGMAX = max(GROUPS)

WINDOWS = [
    (c0, min(COLS_PER_WIN, F - c0)) for c0 in range(0, F, COLS_PER_WIN)
]
NW = len(WINDOWS)


def _patch_queue_lanes():
    """Make Tile assign DMASW sem lanes per SWDGE queue (lane 2q/2q+1 for
    queue q) — the runtime locks each lane to one queue, but stock Tile
    round-robins lanes obliviously."""
    import concourse.tile_sem_assignment as tsa

    if getattr(tsa, "_ffm_queue_patch", False):
        return
    import concourse.mybir as mybir

    orig = tsa.TileClockTick._assign_tick

    def patched(self, inst):
        q = getattr(inst, "queue_num", None)
        if (
            q is not None
            and isinstance(inst, tsa.DMAInst)
            and inst.engine == mybir.EngineType.Pool
        ):
            state = getattr(self, "_ffm_perq", None)
            if state is None:
                state = {}
                self._ffm_perq = state
            self.next_sw_dma_idx = 2 * q + state.get(q, 0)
            orig(self, inst)
            state[q] = state.get(q, 0) ^ 1
            return
        orig(self, inst)

    tsa.TileClockTick._assign_tick = patched
    tsa._ffm_queue_patch = True


def _dma_gather_raw(gp, out_ap, in_ap, idxs_ap, num_idxs, elem_size_bytes,
                    elem_step_bytes, single_packet, queue_num):
    """dma_gather for non-256-multiple elem_size (bass asserts %256==0 as a
    'transpose restriction', but the non-transpose descriptor path moves
    arbitrary byte counts; the row STRIDE still must be a 256 multiple).
    Mirrors BassGpSimd.dma_gather's non-transpose DRAM-source lowering."""
    import concourse.mybir as mybir

    gp._assert_queue_num(queue_num)
    assert idxs_ap.dtype == mybir.dt.int16
    assert in_ap.dtype == out_ap.dtype
    assert elem_step_bytes % 256 == 0
    inst = gp.add_instruction(
        mybir.InstDMAGatherAnt(
            name=gp.bass.get_next_instruction_name(),
            ins=[
                *gp.lower_ap_dma(in_ap, for_custom_bir_dma=True),
                gp.lower_ap(idxs_ap),
                gp.lower_val_access(gp.to_reg(num_idxs)),
            ],
            outs=[gp.lower_ap(out_ap)],
            transpose=False,
            num_idxs=num_idxs,
            elem_size=elem_size_bytes,
            stride_bytes_256=elem_step_bytes // 256,
            gen_mode=0,
            single_packet=single_packet,
            queue_num=queue_num,
            sbuf_tokens_per_rank=0,
            sbuf_free_dim_per_rank=0,
            sbuf_free_dim_pad_per_rank=0,
            sbuf_byte_offset=0,
        )
    )
    return inst


def _build_program(n_tiles=N_TILES, vtot=VTOT, for_sim=False):
    bass, mybir, tile, _ = _import_concourse()
    _patch_queue_lanes()

    v = vtot // F
    gs = GROUPS

    import concourse.bacc as bacc

    # Bacc (not plain Bass): its compile() runs generate_event_semaphores,
    # which splits multi-semaphore waits into InstEventSemaphore prefixes —
    # TRN2 instructions can carry only one inline wait — and inserts the
    # GPSIMD ucode library loads that dma_gather needs.
    nc = bacc.Bacc(None, target_bir_lowering=False, debug=for_sim,
                   num_swdge_queues=N_QUEUES)

    f32 = mybir.dt.float32
    bf16 = mybir.dt.bfloat16
    f8 = mybir.dt.float8e3
    i16 = mybir.dt.int16
    mult = mybir.AluOpType.mult
    copy_f = mybir.ActivationFunctionType.Copy
    sigm_f = mybir.ActivationFunctionType.Sigmoid

    t2 = nc.dram_tensor("t2", [vtot, ROWSTRIDE], f8, kind="ExternalInput")
    idx_cols = sum(ncw * 8 * g for g in gs for (c0, ncw) in WINDOWS)
    idxs = nc.dram_tensor("idxs", [P, idx_cols], i16, kind="ExternalInput")
    dense_d = nc.dram_tensor("dense", [DD + 1, n_tiles, P], f32, kind="ExternalInput")
    wvec_d = nc.dram_tensor("wvec", [DD + 1, 1], f32, kind="ExternalInput")
    out = nc.dram_tensor("out", [P, n_tiles], f32, kind="ExternalOutput")

    with tile.TileContext(nc) as tc:
        with (
            tc.tile_pool(name="const", bufs=1) as cpool,
            tc.tile_pool(name="gather", bufs=3) as gpool,
            tc.tile_pool(name="scratch", bufs=2) as spool,
            tc.tile_pool(name="accp", bufs=4) as apool,
            tc.tile_pool(name="psum", bufs=2, space="PSUM") as pspool,
        ):
            dense_sb = cpool.tile([DD + 1, n_tiles, P], f32)
            wvec_sb = cpool.tile([DD + 1, 1], f32)
            out_all = cpool.tile([P, n_tiles], f32)

            # per-group idx tiles (separate tiles, not slices of one tile —
            # Tile would otherwise make gather 0 wait on every idx DMA),
            # first group's first, so gather 0 isn't gated on the whole
            # index array
            idx_sbs = []
            idx_off0 = 0
            for g_i, ngg in enumerate(gs):
                gcols = ngg * 8 * F
                idx_g = cpool.tile([P, gcols], i16, tag=f"idx{g_i}")
                nc.sync.dma_start(
                    out=idx_g[:],
                    in_=idxs[:, idx_off0 : idx_off0 + gcols],
                )
                idx_sbs.append(idx_g)
                idx_off0 += gcols
            nc.sync.dma_start(out=dense_sb[:], in_=dense_d[:])
            nc.sync.dma_start(out=wvec_sb[:], in_=wvec_d[:])

            # warmup: one tiny gather per queue, no data deps beyond a
            # memset idx — absorbs the first-gather DGE warmup and the
            # startup semaphore serialization before the real gathers
            idxw = cpool.tile([P, 8], i16)
            gwarm = cpool.tile([P, N_QUEUES, ROWB], f8)
            nc.vector.memset(idxw[:], 0)
            for q in range(N_QUEUES):
                _dma_gather_raw(
                    nc.gpsimd,
                    gwarm[:, q : q + 1, :],
                    t2[0:v, :ROWB],
                    idxw[:],
                    P,
                    ROWB,
                    ROWSTRIDE,
                    SINGLE_PACKET,
                    q,
                )

            gather_seq = 0
            tile_base = 0
            for g_i, ngg in enumerate(gs):
                # gather output must be contiguous, so gall is a full tile
                # per group size (not a slice of a shared max-size tile);
                # triple-buffered so a group's gathers never stall on
                # compute two groups back
                gall = gpool.tile([P, F, ngg, ROWB], f8, tag=f"gall{ngg}")
                idx_g = idx_sbs[g_i]
                # reversed window order: the descending-i compute consumes
                # windows last-to-first, so emitting w6 first lets compute
                # start after the first gather of the group lands. idx
                # columns are laid out in emission order within the group.
                idx_off = 0
                for wi, (c0, ncw) in reversed(list(enumerate(WINDOWS))):
                    nidx = ncw * ngg * P
                    ncols = ncw * 8 * ngg
                    _dma_gather_raw(
                        nc.gpsimd,
                        gall[:, c0 : c0 + ncw, :, :].rearrange(
                            "p c n r -> p (c n) r"
                        ),
                        t2[c0 * v : (c0 + ncw) * v, :ROWB],
                        idx_g[:, idx_off : idx_off + ncols],
                        nidx,
                        ROWB,
                        ROWSTRIDE,
                        SINGLE_PACKET,
                        gather_seq % N_QUEUES,
                    )
                    gather_seq += 1
                    idx_off += ncols

                prod_f = spool.tile([P, NGMAX, PRODW], bf16, tag="prod")
                prod = prod_f[:, :ngg, :]

                # pair products for all of the group's tiles at once:
                # per smaller-field i, out[p, n, j, d] =
                #   row_i[block j] * row_j[block i]   (j > i)
                off = 0
                for i in reversed(range(F - 1)):
                    cnt = F - 1 - i
                    x = gall[:, i, :, i * D : EMB].rearrange(
                        "p n (c d) -> p n c d", d=D
                    )
                    y = gall[:, i + 1 : F, :, i * D : (i + 1) * D].rearrange(
                        "p c n d -> p n c d"
                    )
                    nc.vector.tensor_tensor(
                        out=prod[:, :, off : off + cnt * D].rearrange(
                            "p n (c d) -> p n c d", d=D
                        ),
                        in0=x,
                        in1=y,
                        op=mult,
                    )
                    off += cnt * D

                # w_sparse values (bf16, pre-scaled by 1024) behind the
                # products so one reduction covers cross + linear_sparse.
                # scalar_tensor_tensor, NOT tensor_scalar: single-src DVE ops
                # enter 2-port perf mode, which takes the SBUF port pair as an
                # exclusive lock and starves SWDGE descriptor generation
                # (gathers stall); two-tensor ops never contend.
                wsp = gall[:, :, :, WSPB : WSPB + 2].bitcast(bf16).rearrange(
                    "p c n one -> p n (c one)"
                )
                nc.vector.scalar_tensor_tensor(
                    out=prod[:, :, NPAIR_ELEMS:NRED],
                    in0=wsp,
                    scalar=1.0,
                    in1=wsp,
                    op0=mult,
                    op1=mybir.AluOpType.bypass,
                )

                last_group = g_i == len(gs) - 1
                accg = None
                if last_group and ngg > 1:
                    # tail trim: odd tiles reduce on DVE (tensor_reduce never
                    # enters 2-port mode, so it can't starve SWDGE) in
                    # parallel with the even tiles' serial ACT accumulates
                    accg = apool.tile([P, NGMAX], f32, tag="accg")
                    nc.vector.tensor_reduce(
                        out=accg[:, 1:ngg:2],
                        in_=prod[:, 1:ngg:2, :NRED],
                        axis=mybir.AxisListType.X,
                        op=mybir.AluOpType.add,
                    )

                for n in range(ngg):
                    tt = tile_base + n
                    lin = apool.tile([P, 1], f32, tag="lin")

                    if accg is not None and n % 2 == 1:
                        acc = accg[:, n : n + 1]
                    else:
                        acc_t = apool.tile([P, 1], f32, tag="acc")
                        acc = acc_t[:]
                        # cross sum + w_sparse sum: one ACT accumulate
                        # (in-place copy; ACT streams read-then-write)
                        nc.scalar.activation(
                            out=prod[:, n, :NRED],
                            in_=prod[:, n, :NRED],
                            func=copy_f,
                            accum_out=acc,
                        )

                    # linear dense + bias on the (idle) tensor engine
                    ps = pspool.tile([P, 1], f32)
                    nc.tensor.matmul(
                        out=ps[:],
                        lhsT=dense_sb[:, tt, :],
                        rhs=wvec_sb[:, :1],
                        start=True,
                        stop=True,
                    )
                    nc.scalar.copy(out=lin[:], in_=ps[:])

                    # sigmoid(acc/1024 + linear)
                    nc.scalar.activation(
                        out=out_all[:, tt : tt + 1],
                        in_=acc,
                        func=sigm_f,
                        scale=1.0 / (SCALE * SCALE),
                        bias=lin[:],
                    )
                tile_base += ngg

            nc.sync.dma_start(out=out[:], in_=out_all[:])

    nc.compile()
    return nc


_PROGRAM_CACHE = {}


def _get_program():
    if "nc" not in _PROGRAM_CACHE:
        _PROGRAM_CACHE["nc"] = _build_program()
    return _PROGRAM_CACHE["nc"]


def make_idx_array(sparse_core, n_tiles=N_TILES, v=V):
    """sparse_core: [BPC, F] local ids (< V). Returns [P, idx_cols] i16.

    Column layout mirrors _build_program: groups per GROUPS, windows
    emitted in reversed order, idx element i at [partition i%16 (replicated
    8x down), col i//16]; within a gather i = (c_local * ngg + n) * 128 + p.
    """
    gs = GROUPS
    spc = sparse_core.reshape(P, n_tiles, F)  # [p, tt, c], sample s = p*n_tiles+tt
    cols = []
    tile_base = 0
    for ngg in gs:
        for wi, (c0, ncw) in reversed(list(enumerate(WINDOWS))):
            vals = spc[:, tile_base : tile_base + ngg, c0 : c0 + ncw].transpose(
                2, 1, 0
            ).astype(np.int64)
            vals = vals + (np.arange(ncw, dtype=np.int64) * v)[:, None, None]
            flat = vals.reshape(-1).astype(np.int16)
            m = len(flat) // 16
            cols.append(np.tile(flat.reshape(m, 16).T, (8, 1)))  # [128, m]
        tile_base += ngg
    return np.ascontiguousarray(np.concatenate(cols, axis=1))


def _prep_inputs(dense_input, sparse_input, tables, w_dense, w_sparse, bias):
    import ml_dtypes

    dense_input = np.asarray(dense_input, dtype=np.float32)
    sparse_input = np.asarray(sparse_input)
    tables = np.asarray(tables, dtype=np.float32)
    w_dense = np.asarray(w_dense, dtype=np.float32)
    w_sparse = np.asarray(w_sparse, dtype=np.float32)
    bias = np.asarray(bias, dtype=np.float32)

    # T2[g] = [tables[t, g, :]*32 as e3m4 for t != g//V] ++ [w_sparse[g]*1024
    # as bf16] ++ pad
    t2u8 = np.zeros((VTOT, ROWSTRIDE), dtype=np.uint8)
    for c in range(F):
        sl = slice(c * V, (c + 1) * V)
        sel = [t for t in range(F) if t != c]
        emb = tables[sel, sl, :].transpose(1, 0, 2).reshape(V, EMB)
        emb8 = np.clip(emb * SCALE, -CLIP, CLIP).astype(ml_dtypes.float8_e3m4)
        t2u8[sl, :EMB] = emb8.view(np.uint8)
        wspv = (w_sparse[sl, 0] * (SCALE * SCALE)).astype(ml_dtypes.bfloat16)
        t2u8[sl, WSPB : WSPB + 2] = wspv[:, None].view(np.uint8)
    t2 = t2u8.view(ml_dtypes.float8_e3m4)

    sparse_i = sparse_input.astype(np.int64).reshape(N_CORES, BPC, F)
    dense_aug = np.concatenate(
        [dense_input, np.ones((B, 1), dtype=np.float32)], axis=1
    ).reshape(N_CORES, P, N_TILES, DD + 1)
    waug = np.concatenate([w_dense[:, 0], bias]).astype(np.float32)
    wvec = waug.reshape(DD + 1, 1)

    in_maps = []
    for k in range(N_CORES):
        in_maps.append(
            {
                "t2": t2,
                "idxs": make_idx_array(sparse_i[k]),
                "dense": np.ascontiguousarray(dense_aug[k].transpose(2, 1, 0)),
                "wvec": wvec,
            }
        )
    return in_maps


def kernel(dense_input, sparse_input, tables, w_dense, w_sparse, bias, _trace=False):
    _, _, _, bass_utils = _import_concourse()

    nc = _get_program()
    in_maps = _prep_inputs(dense_input, sparse_input, tables, w_dense, w_sparse, bias)
    res = bass_utils.run_bass_kernel_spmd(
        nc, in_maps, core_ids=list(range(N_CORES)), trace=_trace
    )
    outs = [res.results[k]["out"].reshape(BPC) for k in range(N_CORES)]
    full = np.concatenate(outs).reshape(B, 1).astype(np.float32)
    if _trace:
        return full, res
    return full
